# revision 1
# baseline (speedup 1.0000x reference)
"""PointNet++ feature propagation kernel for 8 Trainium2 NeuronCores.

Problem (per batch b of 8, one batch per core):
  1. d2[n,s] = ||xyz1[n] - xyz2[s]||^2            (N=8192, S=2048)
  2. top-3 smallest per n -> idx, dists
  3. w = normalize(1/max(d2,1e-10));  interp[n] = sum_k w_k * points2[idx_k]
  4. X = concat(points1, interp)  [N, 320]
  5. h1 = relu(BN(X @ W1.T)); h2 = relu(BN(h1 @ W2.T))   BN stats over ALL (B,N)
Output: [8, 8192, 128] f32.

Device strategy (per core):
  - s = -d2 via one K=5 fp32 matmul per 128-query tile (coords + folded norms),
    PSUM [128, 2048]; DVE max8 gives top-8 values, max_index their indices.
  - Gather points2 rows with per-partition indirect DMA, scale by weights
    (tensor_scalar 2x mode), transpose-accumulate on PE into interp^T.
  - MLP runs channel-on-partition; BN stats via ACT accum_out; the two tiny
    per-channel stat vectors are AllReduce'd across the 8 cores.
  - Final [C,n] -> [n,C] via PE transpose, DMA out.
"""

import numpy as np

import concourse.bass as bass
import concourse.tile as tile
from concourse import bacc, mybir
from concourse.bass_utils import run_bass_kernel_spmd
from concourse.masks import make_identity

F32 = mybir.dt.float32
U32 = mybir.dt.uint32
AF = mybir.ActivationFunctionType
ALU = mybir.AluOpType

B, N, S, D1, D2 = 8, 8192, 2048, 64, 256
C1, C2 = 256, 128
K = 3
P = 128
NT = N // P  # 64 query tiles
GRP = 4  # tiles per group
NG = NT // GRP  # 16 groups
BN_COUNT = float(B * N)
EPS_BN = 1e-5
EPS_DIST = 1e-10
NCORES = 8


def _build():
    nc = bacc.Bacc("TRN2", target_bir_lowering=False, debug=False)

    A1 = nc.dram_tensor("A1", [5, N], F32, kind="ExternalInput")
    A2 = nc.dram_tensor("A2", [5, S], F32, kind="ExternalInput")
    p1T = nc.dram_tensor("p1T", [D1, N], F32, kind="ExternalInput")
    P2 = nc.dram_tensor("P2", [S, D2], F32, kind="ExternalInput")
    W1T0 = nc.dram_tensor("W1T0", [D1, C1], F32, kind="ExternalInput")
    W1T1 = nc.dram_tensor("W1T1", [128, C1], F32, kind="ExternalInput")
    W1T2 = nc.dram_tensor("W1T2", [128, C1], F32, kind="ExternalInput")
    W2T0 = nc.dram_tensor("W2T0", [128, C2], F32, kind="ExternalInput")
    W2T1 = nc.dram_tensor("W2T1", [128, C2], F32, kind="ExternalInput")
    g1p = nc.dram_tensor("g1p", [P, 2], F32, kind="ExternalInput")
    b1p = nc.dram_tensor("b1p", [P, 2], F32, kind="ExternalInput")
    g2p = nc.dram_tensor("g2p", [P, 1], F32, kind="ExternalInput")
    b2p = nc.dram_tensor("b2p", [P, 1], F32, kind="ExternalInput")
    out_o = nc.dram_tensor("out", [N, C2], F32, kind="ExternalOutput")

    with tile.TileContext(nc) as tc:
        with (
            tc.tile_pool(name="persist", bufs=1) as pp,
            tc.tile_pool(name="work", bufs=2) as pw,
            tc.tile_pool(name="gather", bufs=2) as pg,
            tc.tile_pool(name="dram", bufs=1, space="DRAM") as dr,
        ):
            # ---- static loads -------------------------------------------
            a1sb = pp.tile([5, N], F32, tag="a1sb")
            nc.sync.dma_start(a1sb, A1[:, :])
            a2sb = pp.tile([5, S], F32, tag="a2sb")
            nc.sync.dma_start(a2sb, A2[:, :])
            w10 = pp.tile([D1, C1], F32, tag="w10")
            nc.sync.dma_start(w10, W1T0[:, :])
            w11 = pp.tile([128, C1], F32, tag="w11")
            nc.sync.dma_start(w11, W1T1[:, :])
            w12 = pp.tile([128, C1], F32, tag="w12")
            nc.sync.dma_start(w12, W1T2[:, :])
            w20 = pp.tile([128, C2], F32, tag="w20")
            nc.sync.dma_start(w20, W2T0[:, :])
            w21 = pp.tile([128, C2], F32, tag="w21")
            nc.sync.dma_start(w21, W2T1[:, :])
            g1sb = pp.tile([P, 2], F32, tag="g1sb")
            nc.sync.dma_start(g1sb, g1p[:, :])
            b1sb = pp.tile([P, 2], F32, tag="b1sb")
            nc.sync.dma_start(b1sb, b1p[:, :])
            g2sb = pp.tile([P, 1], F32, tag="g2sb")
            nc.sync.dma_start(g2sb, g2p[:, :])
            b2sb = pp.tile([P, 1], F32, tag="b2sb")
            nc.sync.dma_start(b2sb, b2p[:, :])
            ident = pp.tile([P, P], F32, tag="ident")
            make_identity(nc, ident)

            v_all = pp.tile([P, NT * 8], F32, tag="v_all")
            i_all = pp.tile([P, NT * 8], U32, tag="i_all")
            h1a = pp.tile([P, N], F32, tag="h1a")
            h1b = pp.tile([P, N], F32, tag="h1b")
            h2sb = pp.tile([P, N], F32, tag="h2sb")
            s1sum0 = pp.tile([P, NG], F32, tag="s1sum0")
            s1sum1 = pp.tile([P, NG], F32, tag="s1sum1")
            s1sq0 = pp.tile([P, NG], F32, tag="s1sq0")
            s1sq1 = pp.tile([P, NG], F32, tag="s1sq1")
            s2sum = pp.tile([P, NG], F32, tag="s2sum")
            s2sq = pp.tile([P, NG], F32, tag="s2sq")

            # ---- phase A: distances + top-k -----------------------------
            with tc.tile_pool(name="psA", bufs=2, space="PSUM") as psA:
                for t in range(NT):
                    s_t = psA.tile([P, S], F32, tag="s")
                    lhs = a1sb[:, t * P : (t + 1) * P]
                    for j in range(S // 512):
                        nc.tensor.matmul(
                            s_t[:, j * 512 : (j + 1) * 512],
                            lhs,
                            a2sb[:, j * 512 : (j + 1) * 512],
                            start=True,
                            stop=True,
                        )
                    nc.vector.max(v_all[:, t * 8 : t * 8 + 8], s_t[:, :])
                    nc.vector.max_index(
                        i_all[:, t * 8 : t * 8 + 8], v_all[:, t * 8 : t * 8 + 8], s_t[:, :]
                    )

            # ---- phase B: interpolation weights -------------------------
            v3 = v_all[:, :].rearrange("p (t e) -> p t e", e=8)[:, :, 0:K]
            d3 = pp.tile([P, NT * K], F32, tag="d3")
            d3r = d3[:, :].rearrange("p (t e) -> p t e", e=K)
            nc.vector.tensor_scalar(d3r, v3, -1.0, EPS_DIST, op0=ALU.mult, op1=ALU.max)
            r3 = pp.tile([P, NT * K], F32, tag="r3")
            r3r = r3[:, :].rearrange("p (t e) -> p t e", e=K)
            nc.vector.reciprocal(r3r, d3r)
            rsum = pp.tile([P, NT], F32, tag="rsum")
            nc.vector.reduce_sum(out=rsum[:, :], in_=r3r, axis=mybir.AxisListType.X)
            rsn = pp.tile([P, NT], F32, tag="rsn")
            nc.vector.reciprocal(rsn[:, :], rsum[:, :])
            w3 = pp.tile([P, NT * K], F32, tag="w3")
            w3r = w3[:, :].rearrange("p (t e) -> p t e", e=K)
            rsnb = rsn[:, :].unsqueeze(2).to_broadcast([P, NT, K])
            nc.vector.tensor_tensor(out=w3r, in0=r3r, in1=rsnb, op=ALU.mult)

            # ---- phase C: gather, interp^T, layer-1 matmul + stats ------
            with tc.tile_pool(name="psC", bufs=2, space="PSUM") as psC:
                for g in range(NG):
                    gs = slice(g * GRP * P, (g + 1) * GRP * P)
                    acc0 = psC.tile([P, GRP * P], F32, tag="acc0")
                    acc1 = psC.tile([P, GRP * P], F32, tag="acc1")
                    for u in range(GRP):
                        t = g * GRP + u
                        us = slice(u * P, (u + 1) * P)
                        for k in range(K):
                            gk = pg.tile([P, D2], F32, tag=f"G{k}")
                            nc.gpsimd.indirect_dma_start(
                                out=gk[:],
                                out_offset=None,
                                in_=P2[:, :],
                                in_offset=bass.IndirectOffsetOnAxis(
                                    ap=i_all[:, t * 8 + k : t * 8 + k + 1], axis=0
                                ),
                            )
                            nc.vector.tensor_scalar_mul(
                                gk[:], gk[:], w3[:, t * K + k : t * K + k + 1]
                            )
                            nc.tensor.matmul(
                                acc0[:, us],
                                gk[:, 0:128],
                                ident[:, :],
                                is_transpose=True,
                                start=(k == 0),
                                stop=(k == K - 1),
                            )
                            nc.tensor.matmul(
                                acc1[:, us],
                                gk[:, 128:256],
                                ident[:, :],
                                is_transpose=True,
                                start=(k == 0),
                                stop=(k == K - 1),
                            )
                    xt0 = pw.tile([P, GRP * P], F32, tag="xt0")
                    nc.scalar.copy(xt0, acc0)
                    xt1 = pw.tile([P, GRP * P], F32, tag="xt1")
                    nc.scalar.copy(xt1, acc1)
                    p1g = pw.tile([D1, GRP * P], F32, tag="p1g")
                    nc.sync.dma_start(p1g, p1T[:, gs])
                    for m in range(2):
                        ms = slice(m * 128, (m + 1) * 128)
                        hm = psC.tile([P, GRP * P], F32, tag=f"h1p{m}")
                        nc.tensor.matmul(hm, w10[:, ms], p1g, start=True, stop=False)
                        nc.tensor.matmul(hm, w11[:, ms], xt0, start=False, stop=False)
                        nc.tensor.matmul(hm, w12[:, ms], xt1, start=False, stop=True)
                        dst = h1a if m == 0 else h1b
                        ssum = s1sum0 if m == 0 else s1sum1
                        ssq = s1sq0 if m == 0 else s1sq1
                        nc.scalar.activation(
                            dst[:, gs], hm, AF.Copy, accum_out=ssum[:, g : g + 1]
                        )
                        sq = pw.tile([P, GRP * P], F32, tag="sqscratch")
                        nc.scalar.activation(
                            sq, hm, AF.Square, accum_out=ssq[:, g : g + 1]
                        )

            # ---- BN1 stats allreduce + coefs ----------------------------
            def bn_coefs(sums, sqs, gsb, bsb, ncols, tagp):
                st = pp.tile([P, 2 * ncols], F32, tag=f"{tagp}_st")
                for m in range(ncols):
                    nc.vector.reduce_sum(
                        out=st[:, m : m + 1], in_=sums[m][:, :], axis=mybir.AxisListType.X
                    )
                    nc.vector.reduce_sum(
                        out=st[:, ncols + m : ncols + m + 1],
                        in_=sqs[m][:, :],
                        axis=mybir.AxisListType.X,
                    )
                inb = dr.tile([P, 2 * ncols], F32, tag=f"{tagp}_in")
                outb = dr.tile([P, 2 * ncols], F32, tag=f"{tagp}_out")
                nc.sync.dma_start(inb, st[:, :])
                nc.gpsimd.collective_compute(
                    "AllReduce",
                    ALU.add,
                    replica_groups=[list(range(NCORES))],
                    ins=[inb.opt()],
                    outs=[outb.opt()],
                )
                stg = pp.tile([P, 2 * ncols], F32, tag=f"{tagp}_stg")
                nc.sync.dma_start(stg, outb)
                mu = pp.tile([P, ncols], F32, tag=f"{tagp}_mu")
                nc.vector.tensor_scalar_mul(mu[:, :], stg[:, 0:ncols], 1.0 / BN_COUNT)
                var = pp.tile([P, ncols], F32, tag=f"{tagp}_var")
                nc.vector.tensor_scalar_mul(
                    var[:, :], stg[:, ncols : 2 * ncols], 1.0 / BN_COUNT
                )
                mu2 = pp.tile([P, ncols], F32, tag=f"{tagp}_mu2")
                nc.vector.tensor_tensor(out=mu2[:, :], in0=mu[:, :], in1=mu[:, :], op=ALU.mult)
                nc.vector.tensor_tensor(out=var[:, :], in0=var[:, :], in1=mu2[:, :], op=ALU.subtract)
                nc.vector.tensor_scalar_add(var[:, :], var[:, :], EPS_BN)
                inv = pp.tile([P, ncols], F32, tag=f"{tagp}_inv")
                nc.vector.reciprocal(inv[:, :], var[:, :])
                rst = pp.tile([P, ncols], F32, tag=f"{tagp}_rst")
                nc.scalar.activation(rst[:, :], inv[:, :], AF.Sqrt)
                al = pp.tile([P, ncols], F32, tag=f"{tagp}_al")
                nc.vector.tensor_tensor(out=al[:, :], in0=gsb[:, :], in1=rst[:, :], op=ALU.mult)
                alm = pp.tile([P, ncols], F32, tag=f"{tagp}_alm")
                nc.vector.tensor_tensor(out=alm[:, :], in0=al[:, :], in1=mu[:, :], op=ALU.mult)
                be = pp.tile([P, ncols], F32, tag=f"{tagp}_be")
                nc.vector.tensor_tensor(out=be[:, :], in0=bsb[:, :], in1=alm[:, :], op=ALU.subtract)
                return al, be

            al1, be1 = bn_coefs([s1sum0, s1sum1], [s1sq0, s1sq1], g1sb, b1sb, 2, "bn1")

            # ---- phase D: BN1+relu, layer-2 matmul + stats --------------
            with tc.tile_pool(name="psD", bufs=2, space="PSUM") as psD:
                for g in range(NG):
                    gs = slice(g * GRP * P, (g + 1) * GRP * P)
                    for m, h1m in enumerate((h1a, h1b)):
                        nc.scalar.activation(
                            h1m[:, gs],
                            h1m[:, gs],
                            AF.Relu,
                            bias=be1[:, m : m + 1],
                            scale=al1[:, m : m + 1],
                        )
                    h2p = psD.tile([P, GRP * P], F32, tag="h2p")
                    nc.tensor.matmul(h2p, w20[:, :], h1a[:, gs], start=True, stop=False)
                    nc.tensor.matmul(h2p, w21[:, :], h1b[:, gs], start=False, stop=True)
                    nc.scalar.activation(
                        h2sb[:, gs], h2p, AF.Copy, accum_out=s2sum[:, g : g + 1]
                    )
                    sq2 = pw.tile([P, GRP * P], F32, tag="sq2scratch")
                    nc.scalar.activation(sq2, h2p, AF.Square, accum_out=s2sq[:, g : g + 1])

            al2, be2 = bn_coefs([s2sum], [s2sq], g2sb, b2sb, 1, "bn2")

            # ---- phase E: BN2+relu, transpose to [n, C2], store ---------
            with tc.tile_pool(name="psE", bufs=2, space="PSUM") as psE:
                for t in range(NT):
                    ts_ = slice(t * P, (t + 1) * P)
                    nc.scalar.activation(
                        h2sb[:, ts_],
                        h2sb[:, ts_],
                        AF.Relu,
                        bias=be2[:, 0:1],
                        scale=al2[:, 0:1],
                    )
                    tp = psE.tile([P, P], F32, tag="tp")
                    nc.tensor.matmul(
                        tp, h2sb[:, ts_], ident[:, :], is_transpose=True, start=True, stop=True
                    )
                    ot = pw.tile([P, P], F32, tag="ot")
                    nc.scalar.copy(ot, tp)
                    nc.sync.dma_start(out_o[ts_, :], ot)

    nc.compile()
    return nc


_NC_CACHE = []


def _get_nc():
    if not _NC_CACHE:
        _NC_CACHE.append(_build())
    return _NC_CACHE[0]


def _prep_inputs(xyz1, xyz2, points1, points2, W1, g1, b1, W2, g2, b2):
    xyz1 = np.asarray(xyz1, np.float32)
    xyz2 = np.asarray(xyz2, np.float32)
    points1 = np.asarray(points1, np.float32)
    points2 = np.asarray(points2, np.float32)
    W1 = np.asarray(W1, np.float32)
    W2 = np.asarray(W2, np.float32)
    W1T = np.ascontiguousarray(W1.T)  # [320, 256]
    W2T = np.ascontiguousarray(W2.T)  # [256, 128]
    shared = {
        "W1T0": W1T[0:D1],
        "W1T1": W1T[D1 : D1 + 128],
        "W1T2": W1T[D1 + 128 : D1 + 256],
        "W2T0": W2T[0:128],
        "W2T1": W2T[128:256],
        "g1p": np.ascontiguousarray(np.asarray(g1, np.float32).reshape(2, P).T),
        "b1p": np.ascontiguousarray(np.asarray(b1, np.float32).reshape(2, P).T),
        "g2p": np.ascontiguousarray(np.asarray(g2, np.float32).reshape(1, P).T),
        "b2p": np.ascontiguousarray(np.asarray(b2, np.float32).reshape(1, P).T),
    }
    in_maps = []
    for c in range(NCORES):
        x1 = xyz1[c]
        x2 = xyz2[c]
        A1 = np.empty((5, N), np.float32)
        A1[0:3] = x1.T
        A1[3] = -np.einsum("nc,nc->n", x1, x1)
        A1[4] = 1.0
        A2 = np.empty((5, S), np.float32)
        A2[0:3] = 2.0 * x2.T
        A2[3] = 1.0
        A2[4] = -np.einsum("sc,sc->s", x2, x2)
        in_maps.append(
            {
                "A1": A1,
                "A2": A2,
                "p1T": np.ascontiguousarray(points1[c].T),
                "P2": np.ascontiguousarray(points2[c]),
                **shared,
            }
        )
    return in_maps


def run(inputs, trace=False, trace_kwargs=None):
    in_maps = _prep_inputs(**inputs)
    nc = _get_nc()
    res = run_bass_kernel_spmd(
        nc, in_maps, list(range(NCORES)), trace=trace, **(trace_kwargs or {})
    )
    out = np.stack([res.results[c]["out"] for c in range(NCORES)], axis=0)
    return out.astype(np.float32, copy=False), res


def kernel(**inputs) -> np.ndarray:
    out, _ = run(inputs, trace=False)
    return out


# revision 2
# speedup vs baseline: 1.2247x; 1.2247x over previous
"""PointNet++ feature propagation kernel for 8 Trainium2 NeuronCores.

Problem (per batch b of 8, one batch per core):
  1. d2[n,s] = ||xyz1[n] - xyz2[s]||^2            (N=8192, S=2048)
  2. top-3 smallest per n -> idx, dists
  3. w = normalize(1/max(d2,1e-10));  interp[n] = sum_k w_k * points2[idx_k]
  4. X = concat(points1, interp)  [N, 320]
  5. h1 = relu(BN(X @ W1.T)); h2 = relu(BN(h1 @ W2.T))   BN stats over ALL (B,N)
Output: [8, 8192, 128] f32.

Device strategy (per core):
  - s = -d2 via ONE K=36 bf16 matmul per (128-query tile, 512-col bank):
    3-way bf16 splits of coords and squared norms; bf16 products are exact in
    fp32, so the result is fp32-exact (verified more accurate than a plain
    fp32 matmul in the near-neighbor region) at 8x the fp32 streaming rate.
  - DVE max8 + find_index8 give top-8 values/indices per row (the DVE
    bottleneck, ~4.5us per 128 queries).
  - points2 rows gathered by per-partition indirect DMA, combined with
    interpolation weights on DVE, stored as interp^T-ready bf16 tiles.
  - interp tiles transposed on PE (bf16), layer-1 matmul in fp32,
    layer-2 in bf16; BN stats via ACT accum_out; two tiny per-channel stat
    vectors AllReduce across the 8 cores.
  - All phases share one software-pipelined loop per 4-tile group; PSUM is
    partitioned 4 banks (distance) + 2 (transposes) + 2 (layer-1 out).
"""

import numpy as np
import ml_dtypes

import concourse.bass as bass
import concourse.tile as tile
from concourse import bacc, mybir
from concourse.bass_utils import run_bass_kernel_spmd
from concourse.masks import make_identity

F32 = mybir.dt.float32
BF16 = mybir.dt.bfloat16
U32 = mybir.dt.uint32
AF = mybir.ActivationFunctionType
ALU = mybir.AluOpType
BFNP = ml_dtypes.bfloat16

B, N, S, D1, D2 = 8, 8192, 2048, 64, 256
C1, C2 = 256, 128
K = 3
KD = 36  # distance-matmul contraction rows
P = 128
NT = N // P  # 64 query tiles
GRP = 4  # tiles per group
NG = NT // GRP  # 16 groups
BN_COUNT = float(B * N)
EPS_BN = 1e-5
EPS_DIST = 1e-10
NCORES = 8


def _build():
    nc = bacc.Bacc("TRN2", target_bir_lowering=False, debug=False)

    A1b = nc.dram_tensor("A1b", [KD, N], BF16, kind="ExternalInput")
    A2b = nc.dram_tensor("A2b", [KD, S], BF16, kind="ExternalInput")
    p1T = nc.dram_tensor("p1T", [D1, N], F32, kind="ExternalInput")
    P2 = nc.dram_tensor("P2", [S, D2], F32, kind="ExternalInput")
    W1T0 = nc.dram_tensor("W1T0", [D1, C1], F32, kind="ExternalInput")
    W1T1 = nc.dram_tensor("W1T1", [128, C1], F32, kind="ExternalInput")
    W1T2 = nc.dram_tensor("W1T2", [128, C1], F32, kind="ExternalInput")
    W2T0 = nc.dram_tensor("W2T0", [128, C2], BF16, kind="ExternalInput")
    W2T1 = nc.dram_tensor("W2T1", [128, C2], BF16, kind="ExternalInput")
    g1p = nc.dram_tensor("g1p", [P, 2], F32, kind="ExternalInput")
    b1p = nc.dram_tensor("b1p", [P, 2], F32, kind="ExternalInput")
    g2p = nc.dram_tensor("g2p", [P, 1], F32, kind="ExternalInput")
    b2p = nc.dram_tensor("b2p", [P, 1], F32, kind="ExternalInput")
    out_o = nc.dram_tensor("out", [N, C2], F32, kind="ExternalOutput")

    with tile.TileContext(nc) as tc:
        with (
            tc.tile_pool(name="persist", bufs=1) as pp,
            tc.tile_pool(name="work", bufs=2) as pw,
            tc.tile_pool(name="gather", bufs=2) as pg,
            tc.tile_pool(name="dram", bufs=1, space="DRAM") as dr,
        ):
            # ---- static loads -------------------------------------------
            a1sb = pp.tile([KD, N], BF16, tag="a1sb")
            nc.sync.dma_start(a1sb, A1b[:, :])
            a2sb = pp.tile([KD, S], BF16, tag="a2sb")
            nc.sync.dma_start(a2sb, A2b[:, :])
            w10 = pp.tile([D1, C1], F32, tag="w10")
            nc.sync.dma_start(w10, W1T0[:, :])
            w11 = pp.tile([128, C1], F32, tag="w11")
            nc.sync.dma_start(w11, W1T1[:, :])
            w12 = pp.tile([128, C1], F32, tag="w12")
            nc.sync.dma_start(w12, W1T2[:, :])
            w20 = pp.tile([128, C2], BF16, tag="w20")
            nc.sync.dma_start(w20, W2T0[:, :])
            w21 = pp.tile([128, C2], BF16, tag="w21")
            nc.sync.dma_start(w21, W2T1[:, :])
            g1sb = pp.tile([P, 2], F32, tag="g1sb")
            nc.sync.dma_start(g1sb, g1p[:, :])
            b1sb = pp.tile([P, 2], F32, tag="b1sb")
            nc.sync.dma_start(b1sb, b1p[:, :])
            g2sb = pp.tile([P, 1], F32, tag="g2sb")
            nc.sync.dma_start(g2sb, g2p[:, :])
            b2sb = pp.tile([P, 1], F32, tag="b2sb")
            nc.sync.dma_start(b2sb, b2p[:, :])
            ident = pp.tile([P, P], F32, tag="ident")
            make_identity(nc, ident)
            identb = pp.tile([P, P], BF16, tag="identb")
            nc.vector.tensor_copy(identb, ident)

            v_all = pp.tile([P, NT * 8], F32, tag="v_all")
            i_all = pp.tile([P, NT * 8], U32, tag="i_all")
            d3 = pp.tile([P, NT * K], F32, tag="d3")
            r3 = pp.tile([P, NT * K], F32, tag="r3")
            rsum = pp.tile([P, NT], F32, tag="rsum")
            rsn = pp.tile([P, NT], F32, tag="rsn")
            w3 = pp.tile([P, NT * K], F32, tag="w3")
            interp_all = pp.tile([P, NT * D2], BF16, tag="interp_all")
            h1a = pp.tile([P, N], BF16, tag="h1a")
            h1b = pp.tile([P, N], BF16, tag="h1b")
            h2sb = pp.tile([P, N], F32, tag="h2sb")
            s1sum0 = pp.tile([P, NG], F32, tag="s1sum0")
            s1sum1 = pp.tile([P, NG], F32, tag="s1sum1")
            s1sq0 = pp.tile([P, NG], F32, tag="s1sq0")
            s1sq1 = pp.tile([P, NG], F32, tag="s1sq1")
            s2sum = pp.tile([P, NG], F32, tag="s2sum")
            s2sq = pp.tile([P, NG], F32, tag="s2sq")

            # ---- main pipelined loop ------------------------------------
            with (
                tc.tile_pool(name="psS", bufs=1, space="PSUM") as psS,
                tc.tile_pool(name="psT", bufs=1, space="PSUM") as psT,
                tc.tile_pool(name="psH", bufs=1, space="PSUM") as psH,
            ):
                for g in range(NG):
                    gs = slice(g * GRP * P, (g + 1) * GRP * P)
                    # -- distances + top-k for the 4 tiles of this group
                    for u in range(GRP):
                        t = g * GRP + u
                        s_t = psS.tile([P, S], F32, tag="s")
                        lhs = a1sb[:, t * P : (t + 1) * P]
                        for j in range(S // 512):
                            nc.tensor.matmul(
                                s_t[:, j * 512 : (j + 1) * 512],
                                lhs,
                                a2sb[:, j * 512 : (j + 1) * 512],
                                start=True,
                                stop=True,
                            )
                        nc.vector.max(v_all[:, t * 8 : t * 8 + 8], s_t[:, :])
                        nc.vector.max_index(
                            i_all[:, t * 8 : t * 8 + 8],
                            v_all[:, t * 8 : t * 8 + 8],
                            s_t[:, :],
                        )
                    # -- interpolation weights for the group
                    v3 = v_all[:, g * GRP * 8 : (g + 1) * GRP * 8].rearrange(
                        "p (t e) -> p t e", e=8
                    )[:, :, 0:K]
                    cs = slice(g * GRP * K, (g + 1) * GRP * K)
                    d3r = d3[:, cs].rearrange("p (t e) -> p t e", e=K)
                    nc.vector.tensor_scalar(
                        d3r, v3, -1.0, EPS_DIST, op0=ALU.mult, op1=ALU.max
                    )
                    r3r = r3[:, cs].rearrange("p (t e) -> p t e", e=K)
                    nc.vector.reciprocal(r3r, d3r)
                    rs = slice(g * GRP, (g + 1) * GRP)
                    nc.vector.reduce_sum(
                        out=rsum[:, rs], in_=r3r, axis=mybir.AxisListType.X
                    )
                    nc.vector.reciprocal(rsn[:, rs], rsum[:, rs])
                    w3r = w3[:, cs].rearrange("p (t e) -> p t e", e=K)
                    rsnb = rsn[:, rs].unsqueeze(2).to_broadcast([P, GRP, K])
                    nc.vector.tensor_tensor(out=w3r, in0=r3r, in1=rsnb, op=ALU.mult)
                    # -- gather + weighted combine -> interp (bf16, n-major)
                    for u in range(GRP):
                        t = g * GRP + u
                        tmp = pw.tile([P, D2], F32, tag="ctmp")
                        for k in range(K):
                            gk = pg.tile([P, D2], F32, tag=f"G{k}")
                            nc.gpsimd.indirect_dma_start(
                                out=gk[:],
                                out_offset=None,
                                in_=P2[:, :],
                                in_offset=bass.IndirectOffsetOnAxis(
                                    ap=i_all[:, t * 8 + k : t * 8 + k + 1], axis=0
                                ),
                            )
                            wcol = w3[:, t * K + k : t * K + k + 1]
                            if k == 0:
                                nc.vector.tensor_scalar_mul(tmp[:], gk[:], wcol)
                            elif k == 1:
                                nc.vector.scalar_tensor_tensor(
                                    tmp[:], gk[:], wcol, tmp[:],
                                    op0=ALU.mult, op1=ALU.add,
                                )
                            else:
                                nc.vector.scalar_tensor_tensor(
                                    interp_all[:, t * D2 : (t + 1) * D2],
                                    gk[:], wcol, tmp[:],
                                    op0=ALU.mult, op1=ALU.add,
                                )
                    # -- transpose interp (PE, bf16) and layer-1 matmul
                    tp0 = psT.tile([P, GRP * P], BF16, tag="tp0")
                    tp1 = psT.tile([P, GRP * P], BF16, tag="tp1")
                    for u in range(GRP):
                        t = g * GRP + u
                        us = slice(u * P, (u + 1) * P)
                        nc.tensor.matmul(
                            tp0[:, us],
                            interp_all[:, t * D2 : t * D2 + 128],
                            identb[:, :],
                            is_transpose=True, start=True, stop=True,
                        )
                        nc.tensor.matmul(
                            tp1[:, us],
                            interp_all[:, t * D2 + 128 : (t + 1) * D2],
                            identb[:, :],
                            is_transpose=True, start=True, stop=True,
                        )
                    xt0 = pw.tile([P, GRP * P], F32, tag="xt0")
                    nc.scalar.copy(xt0, tp0)
                    xt1 = pw.tile([P, GRP * P], F32, tag="xt1")
                    nc.scalar.copy(xt1, tp1)
                    p1g = pw.tile([D1, GRP * P], F32, tag="p1g")
                    nc.sync.dma_start(p1g, p1T[:, gs])
                    for m in range(2):
                        ms = slice(m * 128, (m + 1) * 128)
                        hm = psH.tile([P, GRP * P], F32, tag=f"h1p{m}")
                        nc.tensor.matmul(hm, w10[:, ms], p1g, start=True, stop=False)
                        nc.tensor.matmul(hm, w11[:, ms], xt0, start=False, stop=False)
                        nc.tensor.matmul(hm, w12[:, ms], xt1, start=False, stop=True)
                        dst = h1a if m == 0 else h1b
                        ssum = s1sum0 if m == 0 else s1sum1
                        ssq = s1sq0 if m == 0 else s1sq1
                        nc.scalar.activation(
                            dst[:, gs], hm, AF.Copy, accum_out=ssum[:, g : g + 1]
                        )
                        sq = pw.tile([P, GRP * P], F32, tag="sqscratch")
                        nc.scalar.activation(
                            sq, hm, AF.Square, accum_out=ssq[:, g : g + 1]
                        )

            # ---- BN stats allreduce + coefs -----------------------------
            def bn_coefs(sums, sqs, gsb, bsb, ncols, tagp):
                st = pp.tile([P, 2 * ncols], F32, tag=f"{tagp}_st")
                for m in range(ncols):
                    nc.vector.reduce_sum(
                        out=st[:, m : m + 1], in_=sums[m][:, :], axis=mybir.AxisListType.X
                    )
                    nc.vector.reduce_sum(
                        out=st[:, ncols + m : ncols + m + 1],
                        in_=sqs[m][:, :],
                        axis=mybir.AxisListType.X,
                    )
                inb = dr.tile([P, 2 * ncols], F32, tag=f"{tagp}_in")
                outb = dr.tile([P, 2 * ncols], F32, tag=f"{tagp}_out")
                nc.sync.dma_start(inb, st[:, :])
                nc.gpsimd.collective_compute(
                    "AllReduce",
                    ALU.add,
                    replica_groups=[list(range(NCORES))],
                    ins=[inb.opt()],
                    outs=[outb.opt()],
                )
                stg = pp.tile([P, 2 * ncols], F32, tag=f"{tagp}_stg")
                nc.sync.dma_start(stg, outb)
                mu = pp.tile([P, ncols], F32, tag=f"{tagp}_mu")
                nc.vector.tensor_scalar_mul(mu[:, :], stg[:, 0:ncols], 1.0 / BN_COUNT)
                var = pp.tile([P, ncols], F32, tag=f"{tagp}_var")
                nc.vector.tensor_scalar_mul(
                    var[:, :], stg[:, ncols : 2 * ncols], 1.0 / BN_COUNT
                )
                mu2 = pp.tile([P, ncols], F32, tag=f"{tagp}_mu2")
                nc.vector.tensor_tensor(out=mu2[:, :], in0=mu[:, :], in1=mu[:, :], op=ALU.mult)
                nc.vector.tensor_tensor(out=var[:, :], in0=var[:, :], in1=mu2[:, :], op=ALU.subtract)
                nc.vector.tensor_scalar_add(var[:, :], var[:, :], EPS_BN)
                inv = pp.tile([P, ncols], F32, tag=f"{tagp}_inv")
                nc.vector.reciprocal(inv[:, :], var[:, :])
                rst = pp.tile([P, ncols], F32, tag=f"{tagp}_rst")
                nc.scalar.activation(rst[:, :], inv[:, :], AF.Sqrt)
                al = pp.tile([P, ncols], F32, tag=f"{tagp}_al")
                nc.vector.tensor_tensor(out=al[:, :], in0=gsb[:, :], in1=rst[:, :], op=ALU.mult)
                alm = pp.tile([P, ncols], F32, tag=f"{tagp}_alm")
                nc.vector.tensor_tensor(out=alm[:, :], in0=al[:, :], in1=mu[:, :], op=ALU.mult)
                be = pp.tile([P, ncols], F32, tag=f"{tagp}_be")
                nc.vector.tensor_tensor(out=be[:, :], in0=bsb[:, :], in1=alm[:, :], op=ALU.subtract)
                return al, be

            al1, be1 = bn_coefs([s1sum0, s1sum1], [s1sq0, s1sq1], g1sb, b1sb, 2, "bn1")

            # ---- BN1+relu, layer-2 matmul (bf16) + stats ----------------
            with tc.tile_pool(name="psD", bufs=2, space="PSUM") as psD:
                for g in range(NG):
                    gs = slice(g * GRP * P, (g + 1) * GRP * P)
                    for m, h1m in enumerate((h1a, h1b)):
                        nc.scalar.activation(
                            h1m[:, gs],
                            h1m[:, gs],
                            AF.Relu,
                            bias=be1[:, m : m + 1],
                            scale=al1[:, m : m + 1],
                        )
                    h2p = psD.tile([P, GRP * P], F32, tag="h2p")
                    nc.tensor.matmul(h2p, w20[:, :], h1a[:, gs], start=True, stop=False)
                    nc.tensor.matmul(h2p, w21[:, :], h1b[:, gs], start=False, stop=True)
                    nc.scalar.activation(
                        h2sb[:, gs], h2p, AF.Copy, accum_out=s2sum[:, g : g + 1]
                    )
                    sq2 = pw.tile([P, GRP * P], F32, tag="sqscratch")
                    nc.scalar.activation(sq2, h2p, AF.Square, accum_out=s2sq[:, g : g + 1])

            al2, be2 = bn_coefs([s2sum], [s2sq], g2sb, b2sb, 1, "bn2")

            # ---- BN2+relu, transpose to [n, C2], store ------------------
            with tc.tile_pool(name="psE", bufs=2, space="PSUM") as psE:
                for t in range(NT):
                    ts_ = slice(t * P, (t + 1) * P)
                    nc.scalar.activation(
                        h2sb[:, ts_],
                        h2sb[:, ts_],
                        AF.Relu,
                        bias=be2[:, 0:1],
                        scale=al2[:, 0:1],
                    )
                    tp = psE.tile([P, P], F32, tag="tp")
                    nc.tensor.matmul(
                        tp, h2sb[:, ts_], ident[:, :], is_transpose=True, start=True, stop=True
                    )
                    ot = pw.tile([P, P], F32, tag="ot")
                    nc.scalar.copy(ot, tp)
                    nc.sync.dma_start(out_o[ts_, :], ot)

    nc.compile()
    return nc


_NC_CACHE = []


def _get_nc():
    if not _NC_CACHE:
        _NC_CACHE.append(_build())
    return _NC_CACHE[0]


def _split3(v):
    """3-way bf16 split of a float64 array: v ~= a + b + c exactly to ~2^-27."""
    a = v.astype(BFNP).astype(np.float64)
    b = (v - a).astype(BFNP).astype(np.float64)
    c = (v - a - b).astype(BFNP).astype(np.float64)
    return a, b, c


def _dist_rows(x1, x2):
    """Build the K=36 bf16 row pairs computing s = -||x1-x2||^2 fp32-exactly."""
    x = x1.astype(np.float64)
    u2 = x2.astype(np.float64)
    u = 2.0 * u2
    n, s = x.shape[0], u2.shape[0]
    ones_n = np.ones(n)
    ones_s = np.ones(s)
    L, R = [], []
    for i in range(3):
        a, b, c = _split3(x[:, i])
        d, e, f = _split3(u[:, i])
        s1, s2, s3 = _split3(-(x[:, i] ** 2))
        t1, t2, t3 = _split3(-(u2[:, i] ** 2))
        for l, r in [
            (a, d), (s1, ones_s), (ones_n, t1), (s2, ones_s), (ones_n, t2),
            (s3, ones_s), (ones_n, t3), (a, e), (b, d), (a, f), (b, e), (c, d),
        ]:
            L.append(l)
            R.append(r)
    A1b = np.stack(L).astype(BFNP)
    A2b = np.stack(R).astype(BFNP)
    return A1b, A2b


def _prep_inputs(xyz1, xyz2, points1, points2, W1, g1, b1, W2, g2, b2):
    xyz1 = np.asarray(xyz1, np.float32)
    xyz2 = np.asarray(xyz2, np.float32)
    points1 = np.asarray(points1, np.float32)
    points2 = np.asarray(points2, np.float32)
    W1 = np.asarray(W1, np.float32)
    W2 = np.asarray(W2, np.float32)
    W1T = np.ascontiguousarray(W1.T)  # [320, 256]
    W2T = np.ascontiguousarray(W2.T).astype(BFNP)  # [256, 128] bf16
    shared = {
        "W1T0": W1T[0:D1],
        "W1T1": W1T[D1 : D1 + 128],
        "W1T2": W1T[D1 + 128 : D1 + 256],
        "W2T0": W2T[0:128],
        "W2T1": W2T[128:256],
        "g1p": np.ascontiguousarray(np.asarray(g1, np.float32).reshape(2, P).T),
        "b1p": np.ascontiguousarray(np.asarray(b1, np.float32).reshape(2, P).T),
        "g2p": np.ascontiguousarray(np.asarray(g2, np.float32).reshape(1, P).T),
        "b2p": np.ascontiguousarray(np.asarray(b2, np.float32).reshape(1, P).T),
    }
    in_maps = []
    for c in range(NCORES):
        A1b, A2b = _dist_rows(xyz1[c], xyz2[c])
        in_maps.append(
            {
                "A1b": A1b,
                "A2b": A2b,
                "p1T": np.ascontiguousarray(points1[c].T),
                "P2": np.ascontiguousarray(points2[c]),
                **shared,
            }
        )
    return in_maps


def run(inputs, trace=False, trace_kwargs=None):
    in_maps = _prep_inputs(**inputs)
    nc = _get_nc()
    res = run_bass_kernel_spmd(
        nc, in_maps, list(range(NCORES)), trace=trace, **(trace_kwargs or {})
    )
    out = np.stack([res.results[c]["out"] for c in range(NCORES)], axis=0)
    return out.astype(np.float32, copy=False), res


def kernel(**inputs) -> np.ndarray:
    out, _ = run(inputs, trace=False)
    return out


# revision 5
# speedup vs baseline: 1.5003x; 1.2251x over previous
"""PointNet++ feature propagation kernel for 8 Trainium2 NeuronCores.

Problem (per batch b of 8, one batch per core):
  1. d2[n,s] = ||xyz1[n] - xyz2[s]||^2            (N=8192, S=2048)
  2. top-3 smallest per n -> idx, dists
  3. w = normalize(1/max(d2,1e-10));  interp[n] = sum_k w_k * points2[idx_k]
  4. X = concat(points1, interp)  [N, 320]
  5. h1 = relu(BN(X @ W1.T)); h2 = relu(BN(h1 @ W2.T))   BN stats over ALL (B,N)
Output: [8, 8192, 128] f32.

Device dataflow (per core):
  - s = -d2 via ONE K=36 bf16 matmul per (128-query tile, 512-col PSUM bank):
    3-way bf16 splits of coords and squared norms; bf16 products are exact in
    fp32, so the result is fp32-exact at 8x the fp32 streaming rate.
  - DVE max8 + find_index8 per tile (the bottleneck, ~4.5us/tile); s is
    double-buffered in PSUM (2 x 4 banks) so the PE stays ahead of the DVE.
  - points2 (bf16) rows gathered by per-partition indirect DMA right after
    each tile's indices land; the weighted combine runs one group behind so
    gather latency hides under the next tile's top-k.
  - After the scan loop: interp tiles transposed on PE (bf16), layer-1
    matmul bf16, BN stats via ACT accum + DVE square-accum, stats
    AllReduce'd (2KB), BN applies on DVE, layer-2 bf16, second AllReduce,
    final PE transpose to [n, C2] and store.
"""

import numpy as np
import ml_dtypes

import concourse.bass as bass
import concourse.tile as tile
from concourse import bacc, mybir
from concourse.bass_utils import run_bass_kernel_spmd
from concourse.masks import make_identity

F32 = mybir.dt.float32
BF16 = mybir.dt.bfloat16
U32 = mybir.dt.uint32
AF = mybir.ActivationFunctionType
ALU = mybir.AluOpType
BFNP = ml_dtypes.bfloat16

B, N, S, D1, D2 = 8, 8192, 2048, 64, 256
C1, C2 = 256, 128
K = 3
KD = 36  # distance-matmul contraction rows
P = 128
NT = N // P  # 64 query tiles
GRP = 4  # tiles per group
NG = NT // GRP  # 16 groups
BN_COUNT = float(B * N)
EPS_BN = 1e-5
EPS_DIST = 1e-10
NCORES = 8


def _build():
    nc = bacc.Bacc("TRN2", target_bir_lowering=False, debug=False)

    A1b = nc.dram_tensor("A1b", [KD, N], BF16, kind="ExternalInput")
    A2b = nc.dram_tensor("A2b", [KD, S], BF16, kind="ExternalInput")
    p1T = nc.dram_tensor("p1T", [D1, N], BF16, kind="ExternalInput")
    P2 = nc.dram_tensor("P2", [S, D2], BF16, kind="ExternalInput")
    W1T0 = nc.dram_tensor("W1T0", [D1, C1], BF16, kind="ExternalInput")
    W1T1 = nc.dram_tensor("W1T1", [128, C1], BF16, kind="ExternalInput")
    W1T2 = nc.dram_tensor("W1T2", [128, C1], BF16, kind="ExternalInput")
    W2T0 = nc.dram_tensor("W2T0", [128, C2], BF16, kind="ExternalInput")
    W2T1 = nc.dram_tensor("W2T1", [128, C2], BF16, kind="ExternalInput")
    g1p = nc.dram_tensor("g1p", [P, 2], F32, kind="ExternalInput")
    b1p = nc.dram_tensor("b1p", [P, 2], F32, kind="ExternalInput")
    g2p = nc.dram_tensor("g2p", [P, 1], F32, kind="ExternalInput")
    b2p = nc.dram_tensor("b2p", [P, 1], F32, kind="ExternalInput")
    out_o = nc.dram_tensor("out", [N, C2], F32, kind="ExternalOutput")

    with tile.TileContext(nc) as tc:
        with (
            tc.tile_pool(name="persist", bufs=1) as pp,
            tc.tile_pool(name="work", bufs=2) as pw,
            tc.tile_pool(name="gather", bufs=8) as pg,
            tc.tile_pool(name="dram", bufs=1, space="DRAM") as dr,
        ):
            # ---- static loads -------------------------------------------
            a1sb = pp.tile([KD, N], BF16, tag="a1sb")
            nc.sync.dma_start(a1sb, A1b[:, :])
            a2sb = pp.tile([KD, S], BF16, tag="a2sb")
            nc.sync.dma_start(a2sb, A2b[:, :])
            w10 = pp.tile([D1, C1], BF16, tag="w10")
            nc.sync.dma_start(w10, W1T0[:, :])
            w11 = pp.tile([128, C1], BF16, tag="w11")
            nc.sync.dma_start(w11, W1T1[:, :])
            w12 = pp.tile([128, C1], BF16, tag="w12")
            nc.sync.dma_start(w12, W1T2[:, :])
            w20 = pp.tile([128, C2], BF16, tag="w20")
            nc.sync.dma_start(w20, W2T0[:, :])
            w21 = pp.tile([128, C2], BF16, tag="w21")
            nc.sync.dma_start(w21, W2T1[:, :])
            g1sb = pp.tile([P, 2], F32, tag="g1sb")
            nc.sync.dma_start(g1sb, g1p[:, :])
            b1sb = pp.tile([P, 2], F32, tag="b1sb")
            nc.sync.dma_start(b1sb, b1p[:, :])
            g2sb = pp.tile([P, 1], F32, tag="g2sb")
            nc.sync.dma_start(g2sb, g2p[:, :])
            b2sb = pp.tile([P, 1], F32, tag="b2sb")
            nc.sync.dma_start(b2sb, b2p[:, :])
            ident = pp.tile([P, P], F32, tag="ident")
            make_identity(nc, ident)
            identb = pp.tile([P, P], BF16, tag="identb")
            nc.vector.tensor_copy(identb, ident)
            p1sb = pp.tile([D1, N], BF16, tag="p1sb")
            nc.sync.dma_start(p1sb, p1T[:, :])

            v_all = pp.tile([P, NT * 8], F32, tag="v_all")
            i_all = pp.tile([P, NT * 8], U32, tag="i_all")
            d3 = pp.tile([P, NT * K], F32, tag="d3")
            r3 = pp.tile([P, NT * K], F32, tag="r3")
            rsum = pp.tile([P, NT], F32, tag="rsum")
            rsn = pp.tile([P, NT], F32, tag="rsn")
            w3 = pp.tile([P, NT * K], F32, tag="w3")
            interp_all = pp.tile([P, NT * D2], BF16, tag="interp_all")
            h1a = pp.tile([P, N], BF16, tag="h1a")
            h1b = pp.tile([P, N], BF16, tag="h1b")
            h2sb = pp.tile([P, N], F32, tag="h2sb")
            s1sum0 = pp.tile([P, NG], F32, tag="s1sum0")
            s1sum1 = pp.tile([P, NG], F32, tag="s1sum1")
            s1sq0 = pp.tile([P, NG], F32, tag="s1sq0")
            s1sq1 = pp.tile([P, NG], F32, tag="s1sq1")
            s2sum = pp.tile([P, NG], F32, tag="s2sum")
            s2sq = pp.tile([P, NG], F32, tag="s2sq")

            gathered = {}  # (t, k) -> G tile

            def weights_for_group(g):
                v3 = v_all[:, g * GRP * 8 : (g + 1) * GRP * 8].rearrange(
                    "p (t e) -> p t e", e=8
                )[:, :, 0:K]
                cs = slice(g * GRP * K, (g + 1) * GRP * K)
                d3r = d3[:, cs].rearrange("p (t e) -> p t e", e=K)
                nc.vector.tensor_scalar(
                    d3r, v3, -1.0, EPS_DIST, op0=ALU.mult, op1=ALU.max
                )
                r3r = r3[:, cs].rearrange("p (t e) -> p t e", e=K)
                nc.vector.reciprocal(r3r, d3r)
                rs = slice(g * GRP, (g + 1) * GRP)
                nc.vector.reduce_sum(out=rsum[:, rs], in_=r3r, axis=mybir.AxisListType.X)
                nc.vector.reciprocal(rsn[:, rs], rsum[:, rs])
                w3r = w3[:, cs].rearrange("p (t e) -> p t e", e=K)
                rsnb = rsn[:, rs].unsqueeze(2).to_broadcast([P, GRP, K])
                nc.vector.tensor_tensor(out=w3r, in0=r3r, in1=rsnb, op=ALU.mult)

            def combine_group(g):
                for u in range(GRP):
                    t = g * GRP + u
                    tmp = pw.tile([P, D2], BF16, tag="ctmp")
                    for k in range(K):
                        gk = gathered.pop((t, k))
                        wcol = w3[:, t * K + k : t * K + k + 1]
                        if k == 0:
                            nc.vector.tensor_scalar_mul(tmp[:], gk[:], wcol)
                        elif k == 1:
                            nc.vector.scalar_tensor_tensor(
                                tmp[:], gk[:], wcol, tmp[:], op0=ALU.mult, op1=ALU.add
                            )
                        else:
                            nc.vector.scalar_tensor_tensor(
                                interp_all[:, t * D2 : (t + 1) * D2],
                                gk[:], wcol, tmp[:], op0=ALU.mult, op1=ALU.add,
                            )

            # ---- scan loop: distances, top-k, gathers, combine ----------
            with tc.tile_pool(name="psS", bufs=2, space="PSUM") as psS:
                for g in range(NG):
                    for u in range(GRP):
                        t = g * GRP + u
                        s_t = psS.tile([P, S], F32, tag="s")
                        lhs = a1sb[:, t * P : (t + 1) * P]
                        for j in range(S // 512):
                            nc.tensor.matmul(
                                s_t[:, j * 512 : (j + 1) * 512],
                                lhs,
                                a2sb[:, j * 512 : (j + 1) * 512],
                                start=True,
                                stop=True,
                            )
                        nc.vector.max(v_all[:, t * 8 : t * 8 + 8], s_t[:, :])
                        nc.vector.max_index(
                            i_all[:, t * 8 : t * 8 + 8],
                            v_all[:, t * 8 : t * 8 + 8],
                            s_t[:, :],
                        )
                        for k in range(K):
                            gk = pg.tile([P, D2], BF16, tag=f"G{k}")
                            nc.gpsimd.indirect_dma_start(
                                out=gk[:],
                                out_offset=None,
                                in_=P2[:, :],
                                in_offset=bass.IndirectOffsetOnAxis(
                                    ap=i_all[:, t * 8 + k : t * 8 + k + 1], axis=0
                                ),
                            )
                            gathered[(t, k)] = gk
                    weights_for_group(g)
                    if g > 0:
                        combine_group(g - 1)
                combine_group(NG - 1)

            # ---- interp transpose + layer-1 matmul + stats --------------
            with (
                tc.tile_pool(name="psT", bufs=2, space="PSUM") as psT,
                tc.tile_pool(name="psH", bufs=2, space="PSUM") as psH,
            ):
                for g in range(NG):
                    gs = slice(g * GRP * P, (g + 1) * GRP * P)
                    tp0 = psT.tile([P, GRP * P], BF16, tag="tp0")
                    tp1 = psT.tile([P, GRP * P], BF16, tag="tp1")
                    for u in range(GRP):
                        t = g * GRP + u
                        us = slice(u * P, (u + 1) * P)
                        nc.tensor.matmul(
                            tp0[:, us],
                            interp_all[:, t * D2 : t * D2 + 128],
                            identb[:, :],
                            is_transpose=True, start=True, stop=True,
                        )
                        nc.tensor.matmul(
                            tp1[:, us],
                            interp_all[:, t * D2 + 128 : (t + 1) * D2],
                            identb[:, :],
                            is_transpose=True, start=True, stop=True,
                        )
                    xt0 = pw.tile([P, GRP * P], BF16, tag="xt0")
                    nc.scalar.copy(xt0, tp0)
                    xt1 = pw.tile([P, GRP * P], BF16, tag="xt1")
                    nc.scalar.copy(xt1, tp1)
                    for m in range(2):
                        ms = slice(m * 128, (m + 1) * 128)
                        hm = psH.tile([P, GRP * P], F32, tag=f"h1p{m}")
                        nc.tensor.matmul(hm, w10[:, ms], p1sb[:, gs], start=True, stop=False)
                        nc.tensor.matmul(hm, w11[:, ms], xt0, start=False, stop=False)
                        nc.tensor.matmul(hm, w12[:, ms], xt1, start=False, stop=True)
                        dst = h1a if m == 0 else h1b
                        ssum = s1sum0 if m == 0 else s1sum1
                        ssq = s1sq0 if m == 0 else s1sq1
                        nc.scalar.activation(
                            dst[:, gs], hm, AF.Copy, accum_out=ssum[:, g : g + 1]
                        )
                        sq = pw.tile([P, GRP * P], F32, tag="sqscratch")
                        nc.scalar.activation(
                            sq, hm, AF.Square, accum_out=ssq[:, g : g + 1]
                        )

            # ---- BN stats allreduce + coefs -----------------------------
            def bn_coefs(sums, sqs, gsb, bsb, ncols, tagp):
                st = pp.tile([P, 2 * ncols], F32, tag=f"{tagp}_st")
                for m in range(ncols):
                    nc.vector.reduce_sum(
                        out=st[:, m : m + 1], in_=sums[m][:, :], axis=mybir.AxisListType.X
                    )
                    nc.vector.reduce_sum(
                        out=st[:, ncols + m : ncols + m + 1],
                        in_=sqs[m][:, :],
                        axis=mybir.AxisListType.X,
                    )
                inb = dr.tile([P, 2 * ncols], F32, tag=f"{tagp}_in")
                outb = dr.tile([P, 2 * ncols], F32, tag=f"{tagp}_out")
                nc.sync.dma_start(inb, st[:, :])
                nc.gpsimd.collective_compute(
                    "AllReduce",
                    ALU.add,
                    replica_groups=[list(range(NCORES))],
                    ins=[inb.opt()],
                    outs=[outb.opt()],
                )
                stg = pp.tile([P, 2 * ncols], F32, tag=f"{tagp}_stg")
                nc.sync.dma_start(stg, outb)
                mu = pp.tile([P, ncols], F32, tag=f"{tagp}_mu")
                nc.vector.tensor_scalar_mul(mu[:, :], stg[:, 0:ncols], 1.0 / BN_COUNT)
                var = pp.tile([P, ncols], F32, tag=f"{tagp}_var")
                nc.vector.tensor_scalar_mul(
                    var[:, :], stg[:, ncols : 2 * ncols], 1.0 / BN_COUNT
                )
                mu2 = pp.tile([P, ncols], F32, tag=f"{tagp}_mu2")
                nc.vector.tensor_tensor(out=mu2[:, :], in0=mu[:, :], in1=mu[:, :], op=ALU.mult)
                nc.vector.tensor_tensor(out=var[:, :], in0=var[:, :], in1=mu2[:, :], op=ALU.subtract)
                nc.vector.tensor_scalar_add(var[:, :], var[:, :], EPS_BN)
                inv = pp.tile([P, ncols], F32, tag=f"{tagp}_inv")
                nc.vector.reciprocal(inv[:, :], var[:, :])
                rst = pp.tile([P, ncols], F32, tag=f"{tagp}_rst")
                nc.scalar.activation(rst[:, :], inv[:, :], AF.Sqrt)
                al = pp.tile([P, ncols], F32, tag=f"{tagp}_al")
                nc.vector.tensor_tensor(out=al[:, :], in0=gsb[:, :], in1=rst[:, :], op=ALU.mult)
                alm = pp.tile([P, ncols], F32, tag=f"{tagp}_alm")
                nc.vector.tensor_tensor(out=alm[:, :], in0=al[:, :], in1=mu[:, :], op=ALU.mult)
                be = pp.tile([P, ncols], F32, tag=f"{tagp}_be")
                nc.vector.tensor_tensor(out=be[:, :], in0=bsb[:, :], in1=alm[:, :], op=ALU.subtract)
                return al, be

            al1, be1 = bn_coefs([s1sum0, s1sum1], [s1sq0, s1sq1], g1sb, b1sb, 2, "bn1")

            # ---- BN1+relu (DVE), layer-2 matmul (bf16) + stats ----------
            with tc.tile_pool(name="psD", bufs=2, space="PSUM") as psD:
                for g in range(NG):
                    gs = slice(g * GRP * P, (g + 1) * GRP * P)
                    for m, h1m in enumerate((h1a, h1b)):
                        nc.vector.tensor_scalar(
                            h1m[:, gs], h1m[:, gs],
                            al1[:, m : m + 1], be1[:, m : m + 1],
                            op0=ALU.mult, op1=ALU.add,
                        )
                        nc.vector.tensor_scalar_max(h1m[:, gs], h1m[:, gs], 0.0)
                    h2p = psD.tile([P, GRP * P], F32, tag="h2p")
                    nc.tensor.matmul(h2p, w20[:, :], h1a[:, gs], start=True, stop=False)
                    nc.tensor.matmul(h2p, w21[:, :], h1b[:, gs], start=False, stop=True)
                    nc.scalar.activation(
                        h2sb[:, gs], h2p, AF.Copy, accum_out=s2sum[:, g : g + 1]
                    )
                    sq2 = pw.tile([P, GRP * P], F32, tag="sqscratch")
                    nc.scalar.activation(
                        sq2, h2p, AF.Square, accum_out=s2sq[:, g : g + 1]
                    )

            al2, be2 = bn_coefs([s2sum], [s2sq], g2sb, b2sb, 1, "bn2")

            # ---- BN2+relu (DVE), transpose to [n, C2], store ------------
            with tc.tile_pool(name="psE", bufs=2, space="PSUM") as psE:
                for g in range(NG):
                    gs = slice(g * GRP * P, (g + 1) * GRP * P)
                    nc.vector.tensor_scalar(
                        h2sb[:, gs], h2sb[:, gs],
                        al2[:, 0:1], be2[:, 0:1],
                        op0=ALU.mult, op1=ALU.add,
                    )
                    nc.vector.tensor_scalar_max(h2sb[:, gs], h2sb[:, gs], 0.0)
                for t in range(NT):
                    ts_ = slice(t * P, (t + 1) * P)
                    tp = psE.tile([P, P], F32, tag="tp")
                    nc.tensor.matmul(
                        tp, h2sb[:, ts_], ident[:, :], is_transpose=True, start=True, stop=True
                    )
                    ot = pw.tile([P, P], F32, tag="ot")
                    nc.scalar.copy(ot, tp)
                    nc.sync.dma_start(out_o[ts_, :], ot)

    nc.compile()
    return nc


_NC_CACHE = []


def _get_nc():
    if not _NC_CACHE:
        _NC_CACHE.append(_build())
    return _NC_CACHE[0]


def _split3(v):
    """3-way bf16 split of a float64 array: v ~= a + b + c exactly to ~2^-27."""
    a = v.astype(BFNP).astype(np.float64)
    b = (v - a).astype(BFNP).astype(np.float64)
    c = (v - a - b).astype(BFNP).astype(np.float64)
    return a, b, c


def _dist_rows(x1, x2):
    """Build the K=36 bf16 row pairs computing s = -||x1-x2||^2 fp32-exactly."""
    x = x1.astype(np.float64)
    u2 = x2.astype(np.float64)
    u = 2.0 * u2
    n, s = x.shape[0], u2.shape[0]
    ones_n = np.ones(n)
    ones_s = np.ones(s)
    L, R = [], []
    for i in range(3):
        a, b, c = _split3(x[:, i])
        d, e, f = _split3(u[:, i])
        s1, s2, s3 = _split3(-(x[:, i] ** 2))
        t1, t2, t3 = _split3(-(u2[:, i] ** 2))
        for l, r in [
            (a, d), (s1, ones_s), (ones_n, t1), (s2, ones_s), (ones_n, t2),
            (s3, ones_s), (ones_n, t3), (a, e), (b, d), (a, f), (b, e), (c, d),
        ]:
            L.append(l)
            R.append(r)
    A1b = np.stack(L).astype(BFNP)
    A2b = np.stack(R).astype(BFNP)
    return A1b, A2b


def _prep_inputs(xyz1, xyz2, points1, points2, W1, g1, b1, W2, g2, b2):
    xyz1 = np.asarray(xyz1, np.float32)
    xyz2 = np.asarray(xyz2, np.float32)
    points1 = np.asarray(points1, np.float32)
    points2 = np.asarray(points2, np.float32)
    W1 = np.asarray(W1, np.float32)
    W2 = np.asarray(W2, np.float32)
    W1T = np.ascontiguousarray(W1.T).astype(BFNP)  # [320, 256]
    W2T = np.ascontiguousarray(W2.T).astype(BFNP)  # [256, 128]
    shared = {
        "W1T0": W1T[0:D1],
        "W1T1": W1T[D1 : D1 + 128],
        "W1T2": W1T[D1 + 128 : D1 + 256],
        "W2T0": W2T[0:128],
        "W2T1": W2T[128:256],
        "g1p": np.ascontiguousarray(np.asarray(g1, np.float32).reshape(2, P).T),
        "b1p": np.ascontiguousarray(np.asarray(b1, np.float32).reshape(2, P).T),
        "g2p": np.ascontiguousarray(np.asarray(g2, np.float32).reshape(1, P).T),
        "b2p": np.ascontiguousarray(np.asarray(b2, np.float32).reshape(1, P).T),
    }
    in_maps = []
    for c in range(NCORES):
        A1b, A2b = _dist_rows(xyz1[c], xyz2[c])
        in_maps.append(
            {
                "A1b": A1b,
                "A2b": A2b,
                "p1T": np.ascontiguousarray(points1[c].T).astype(BFNP),
                "P2": np.ascontiguousarray(points2[c]).astype(BFNP),
                **shared,
            }
        )
    return in_maps


def run(inputs, trace=False, trace_kwargs=None):
    in_maps = _prep_inputs(**inputs)
    nc = _get_nc()
    res = run_bass_kernel_spmd(
        nc, in_maps, list(range(NCORES)), trace=trace, **(trace_kwargs or {})
    )
    out = np.stack([res.results[c]["out"] for c in range(NCORES)], axis=0)
    return out.astype(np.float32, copy=False), res


def kernel(**inputs) -> np.ndarray:
    out, _ = run(inputs, trace=False)
    return out


# revision 6
# speedup vs baseline: 1.8544x; 1.2360x over previous
"""PointNet++ feature propagation kernel for 8 Trainium2 NeuronCores.

Problem (per batch b of 8, one batch per core):
  1. d2[n,s] = ||xyz1[n] - xyz2[s]||^2            (N=8192, S=2048)
  2. top-3 smallest per n -> idx, dists
  3. w = normalize(1/max(d2,1e-10));  interp[n] = sum_k w_k * points2[idx_k]
  4. X = concat(points1, interp)  [N, 320]
  5. h1 = relu(BN(X @ W1.T)); h2 = relu(BN(h1 @ W2.T))   BN stats over ALL (B,N)
Output: [8, 8192, 128] f32 (device produces channel-major; host transposes).

Device dataflow (per core):
  - s = -d2 via ONE K=36 bf16 matmul per (128-query tile, 512-col PSUM bank):
    3-way bf16 splits of coords and squared norms; bf16 products are exact in
    fp32, so the result is fp32-exact at 8x the fp32 streaming rate.
  - DVE max8 + find_index8 per tile (the bottleneck, ~4.5us/tile); s is
    double-buffered in PSUM (2 x 4 banks). Indices land in per-tile rotating
    buffers so the gather's descriptor-gen never blocks the next find_index.
  - points2 (bf16) rows gathered via per-partition indirect DMA; ACT does the
    per-neighbor weight scaling (Copy activation with per-partition scale),
    DVE just adds the three scaled tiles.
  - Post-loop: interp transposed on PE (bf16), layer-1 matmul bf16, stats via
    ACT/DVE accum_out, 2KB AllReduce, BN applies on DVE, layer-2 bf16, second
    AllReduce, then the output streams out channel-major (no final transpose).
"""

import numpy as np
import ml_dtypes

import concourse.bass as bass
import concourse.tile as tile
from concourse import bacc, mybir
from concourse.bass_utils import run_bass_kernel_spmd
from concourse.masks import make_identity

F32 = mybir.dt.float32
BF16 = mybir.dt.bfloat16
U32 = mybir.dt.uint32
AF = mybir.ActivationFunctionType
ALU = mybir.AluOpType
BFNP = ml_dtypes.bfloat16

B, N, S, D1, D2 = 8, 8192, 2048, 64, 256
C1, C2 = 256, 128
K = 3
KD = 36  # distance-matmul contraction rows
P = 128
NT = N // P  # 64 query tiles
GRP = 4  # tiles per group
NG = NT // GRP  # 16 groups
BN_COUNT = float(B * N)
EPS_BN = 1e-5
EPS_DIST = 1e-10
NCORES = 8


def _build():
    nc = bacc.Bacc("TRN2", target_bir_lowering=False, debug=False)

    A1b = nc.dram_tensor("A1b", [KD, N], BF16, kind="ExternalInput")
    A2b = nc.dram_tensor("A2b", [KD, S], BF16, kind="ExternalInput")
    p1T = nc.dram_tensor("p1T", [D1, N], BF16, kind="ExternalInput")
    P2 = nc.dram_tensor("P2", [S, D2], BF16, kind="ExternalInput")
    W1T0 = nc.dram_tensor("W1T0", [D1, C1], BF16, kind="ExternalInput")
    W1T1 = nc.dram_tensor("W1T1", [128, C1], BF16, kind="ExternalInput")
    W1T2 = nc.dram_tensor("W1T2", [128, C1], BF16, kind="ExternalInput")
    W2T0 = nc.dram_tensor("W2T0", [128, C2], BF16, kind="ExternalInput")
    W2T1 = nc.dram_tensor("W2T1", [128, C2], BF16, kind="ExternalInput")
    g1p = nc.dram_tensor("g1p", [P, 2], F32, kind="ExternalInput")
    b1p = nc.dram_tensor("b1p", [P, 2], F32, kind="ExternalInput")
    g2p = nc.dram_tensor("g2p", [P, 1], F32, kind="ExternalInput")
    b2p = nc.dram_tensor("b2p", [P, 1], F32, kind="ExternalInput")
    out_o = nc.dram_tensor("out", [C2, N], F32, kind="ExternalOutput")

    with tile.TileContext(nc) as tc:
        with (
            tc.tile_pool(name="persist", bufs=1) as pp,
            tc.tile_pool(name="work", bufs=2) as pw,
            tc.tile_pool(name="gather", bufs=8) as pg,
            tc.tile_pool(name="idxp", bufs=8) as pidx,
            tc.tile_pool(name="dram", bufs=1, space="DRAM") as dr,
        ):
            # ---- static loads -------------------------------------------
            a1sb = pp.tile([KD, N], BF16, tag="a1sb")
            nc.sync.dma_start(a1sb, A1b[:, :])
            a2sb = pp.tile([KD, S], BF16, tag="a2sb")
            nc.sync.dma_start(a2sb, A2b[:, :])
            w10 = pp.tile([D1, C1], BF16, tag="w10")
            nc.sync.dma_start(w10, W1T0[:, :])
            w11 = pp.tile([128, C1], BF16, tag="w11")
            nc.sync.dma_start(w11, W1T1[:, :])
            w12 = pp.tile([128, C1], BF16, tag="w12")
            nc.sync.dma_start(w12, W1T2[:, :])
            w20 = pp.tile([128, C2], BF16, tag="w20")
            nc.sync.dma_start(w20, W2T0[:, :])
            w21 = pp.tile([128, C2], BF16, tag="w21")
            nc.sync.dma_start(w21, W2T1[:, :])
            g1sb = pp.tile([P, 2], F32, tag="g1sb")
            nc.sync.dma_start(g1sb, g1p[:, :])
            b1sb = pp.tile([P, 2], F32, tag="b1sb")
            nc.sync.dma_start(b1sb, b1p[:, :])
            g2sb = pp.tile([P, 1], F32, tag="g2sb")
            nc.sync.dma_start(g2sb, g2p[:, :])
            b2sb = pp.tile([P, 1], F32, tag="b2sb")
            nc.sync.dma_start(b2sb, b2p[:, :])
            identb = pp.tile([P, P], BF16, tag="identb")
            make_identity(nc, identb)
            p1sb = pp.tile([D1, N], BF16, tag="p1sb")
            nc.sync.dma_start(p1sb, p1T[:, :])

            v_all = pp.tile([P, NT * 8], F32, tag="v_all")
            d3 = pp.tile([P, NT * K], F32, tag="d3")
            r3 = pp.tile([P, NT * K], F32, tag="r3")
            rsum = pp.tile([P, NT], F32, tag="rsum")
            rsn = pp.tile([P, NT], F32, tag="rsn")
            w3 = pp.tile([P, NT * K], F32, tag="w3")
            interp_all = pp.tile([P, NT * D2], BF16, tag="interp_all")
            h1a = pp.tile([P, N], BF16, tag="h1a")
            h1b = pp.tile([P, N], BF16, tag="h1b")
            h2sb = pp.tile([P, N], F32, tag="h2sb")
            s1sum0 = pp.tile([P, NG], F32, tag="s1sum0")
            s1sum1 = pp.tile([P, NG], F32, tag="s1sum1")
            s1sq0 = pp.tile([P, NG], F32, tag="s1sq0")
            s1sq1 = pp.tile([P, NG], F32, tag="s1sq1")
            s2sum = pp.tile([P, NG], F32, tag="s2sum")
            s2sq = pp.tile([P, NG], F32, tag="s2sq")

            gathered = {}  # (t, k) -> G tile

            def weights_for_group(g):
                v3 = v_all[:, g * GRP * 8 : (g + 1) * GRP * 8].rearrange(
                    "p (t e) -> p t e", e=8
                )[:, :, 0:K]
                cs = slice(g * GRP * K, (g + 1) * GRP * K)
                d3r = d3[:, cs].rearrange("p (t e) -> p t e", e=K)
                nc.vector.tensor_scalar(
                    d3r, v3, -1.0, EPS_DIST, op0=ALU.mult, op1=ALU.max
                )
                r3r = r3[:, cs].rearrange("p (t e) -> p t e", e=K)
                nc.vector.reciprocal(r3r, d3r)
                rs = slice(g * GRP, (g + 1) * GRP)
                nc.vector.reduce_sum(out=rsum[:, rs], in_=r3r, axis=mybir.AxisListType.X)
                nc.vector.reciprocal(rsn[:, rs], rsum[:, rs])
                w3r = w3[:, cs].rearrange("p (t e) -> p t e", e=K)
                rsnb = rsn[:, rs].unsqueeze(2).to_broadcast([P, GRP, K])
                nc.vector.tensor_tensor(out=w3r, in0=r3r, in1=rsnb, op=ALU.mult)

            def combine_group(g):
                for u in range(GRP):
                    t = g * GRP + u
                    scs = []
                    for k in range(K):
                        gk = gathered.pop((t, k))
                        sck = pw.tile([P, D2], BF16, tag=f"sc{k}")
                        nc.scalar.activation(
                            sck, gk, AF.Copy,
                            scale=w3[:, t * K + k : t * K + k + 1],
                        )
                        scs.append(sck)
                    tmp = pw.tile([P, D2], BF16, tag="ctmp")
                    nc.vector.tensor_tensor(out=tmp, in0=scs[0], in1=scs[1], op=ALU.add)
                    nc.vector.tensor_tensor(
                        out=interp_all[:, t * D2 : (t + 1) * D2],
                        in0=tmp, in1=scs[2], op=ALU.add,
                    )

            # ---- scan loop: distances, top-k, gathers, combine ----------
            with tc.tile_pool(name="psS", bufs=2, space="PSUM") as psS:
                for g in range(NG):
                    for u in range(GRP):
                        t = g * GRP + u
                        s_t = psS.tile([P, S], F32, tag="s")
                        lhs = a1sb[:, t * P : (t + 1) * P]
                        for j in range(S // 512):
                            nc.tensor.matmul(
                                s_t[:, j * 512 : (j + 1) * 512],
                                lhs,
                                a2sb[:, j * 512 : (j + 1) * 512],
                                start=True,
                                stop=True,
                            )
                        nc.vector.max(v_all[:, t * 8 : t * 8 + 8], s_t[:, :])
                        it = pidx.tile([P, 8], U32, tag="idx")
                        nc.vector.max_index(
                            it[:, :], v_all[:, t * 8 : t * 8 + 8], s_t[:, :]
                        )
                        for k in range(K):
                            gk = pg.tile([P, D2], BF16, tag=f"G{k}")
                            nc.gpsimd.indirect_dma_start(
                                out=gk[:],
                                out_offset=None,
                                in_=P2[:, :],
                                in_offset=bass.IndirectOffsetOnAxis(
                                    ap=it[:, k : k + 1], axis=0
                                ),
                            )
                            gathered[(t, k)] = gk
                    weights_for_group(g)
                    if g > 0:
                        combine_group(g - 1)
                combine_group(NG - 1)

            # ---- interp transpose + layer-1 matmul + stats --------------
            with (
                tc.tile_pool(name="psT", bufs=2, space="PSUM") as psT,
                tc.tile_pool(name="psH", bufs=2, space="PSUM") as psH,
            ):
                for g in range(NG):
                    gs = slice(g * GRP * P, (g + 1) * GRP * P)
                    tp0 = psT.tile([P, GRP * P], BF16, tag="tp0")
                    tp1 = psT.tile([P, GRP * P], BF16, tag="tp1")
                    for u in range(GRP):
                        t = g * GRP + u
                        us = slice(u * P, (u + 1) * P)
                        nc.tensor.matmul(
                            tp0[:, us],
                            interp_all[:, t * D2 : t * D2 + 128],
                            identb[:, :],
                            is_transpose=True, start=True, stop=True,
                        )
                        nc.tensor.matmul(
                            tp1[:, us],
                            interp_all[:, t * D2 + 128 : (t + 1) * D2],
                            identb[:, :],
                            is_transpose=True, start=True, stop=True,
                        )
                    xt0 = pw.tile([P, GRP * P], BF16, tag="xt0")
                    nc.vector.tensor_copy(xt0, tp0)
                    xt1 = pw.tile([P, GRP * P], BF16, tag="xt1")
                    nc.vector.tensor_copy(xt1, tp1)
                    for m in range(2):
                        ms = slice(m * 128, (m + 1) * 128)
                        hm = psH.tile([P, GRP * P], F32, tag=f"h1p{m}")
                        nc.tensor.matmul(hm, w10[:, ms], p1sb[:, gs], start=True, stop=False)
                        nc.tensor.matmul(hm, w11[:, ms], xt0, start=False, stop=False)
                        nc.tensor.matmul(hm, w12[:, ms], xt1, start=False, stop=True)
                        dst = h1a if m == 0 else h1b
                        ssum = s1sum0 if m == 0 else s1sum1
                        ssq = s1sq0 if m == 0 else s1sq1
                        nc.scalar.activation(
                            dst[:, gs], hm, AF.Copy, accum_out=ssum[:, g : g + 1]
                        )
                        sq = pw.tile([P, GRP * P], F32, tag="sqscratch")
                        nc.scalar.activation(
                            sq, hm, AF.Square, accum_out=ssq[:, g : g + 1]
                        )

            # ---- BN stats allreduce + coefs -----------------------------
            def bn_coefs(sums, sqs, gsb, bsb, ncols, tagp):
                st = pp.tile([P, 2 * ncols], F32, tag=f"{tagp}_st")
                for m in range(ncols):
                    nc.vector.reduce_sum(
                        out=st[:, m : m + 1], in_=sums[m][:, :], axis=mybir.AxisListType.X
                    )
                    nc.vector.reduce_sum(
                        out=st[:, ncols + m : ncols + m + 1],
                        in_=sqs[m][:, :],
                        axis=mybir.AxisListType.X,
                    )
                inb = dr.tile([P, 2 * ncols], F32, tag=f"{tagp}_in")
                outb = dr.tile([P, 2 * ncols], F32, tag=f"{tagp}_out")
                nc.sync.dma_start(inb, st[:, :])
                nc.gpsimd.collective_compute(
                    "AllReduce",
                    ALU.add,
                    replica_groups=[list(range(NCORES))],
                    ins=[inb.opt()],
                    outs=[outb.opt()],
                )
                stg = pp.tile([P, 2 * ncols], F32, tag=f"{tagp}_stg")
                nc.sync.dma_start(stg, outb)
                mu = pp.tile([P, ncols], F32, tag=f"{tagp}_mu")
                nc.vector.tensor_scalar_mul(mu[:, :], stg[:, 0:ncols], 1.0 / BN_COUNT)
                var = pp.tile([P, ncols], F32, tag=f"{tagp}_var")
                nc.vector.tensor_scalar_mul(
                    var[:, :], stg[:, ncols : 2 * ncols], 1.0 / BN_COUNT
                )
                mu2 = pp.tile([P, ncols], F32, tag=f"{tagp}_mu2")
                nc.vector.tensor_tensor(out=mu2[:, :], in0=mu[:, :], in1=mu[:, :], op=ALU.mult)
                nc.vector.tensor_tensor(out=var[:, :], in0=var[:, :], in1=mu2[:, :], op=ALU.subtract)
                nc.vector.tensor_scalar_add(var[:, :], var[:, :], EPS_BN)
                inv = pp.tile([P, ncols], F32, tag=f"{tagp}_inv")
                nc.vector.reciprocal(inv[:, :], var[:, :])
                rst = pp.tile([P, ncols], F32, tag=f"{tagp}_rst")
                nc.scalar.activation(rst[:, :], inv[:, :], AF.Sqrt)
                al = pp.tile([P, ncols], F32, tag=f"{tagp}_al")
                nc.vector.tensor_tensor(out=al[:, :], in0=gsb[:, :], in1=rst[:, :], op=ALU.mult)
                alm = pp.tile([P, ncols], F32, tag=f"{tagp}_alm")
                nc.vector.tensor_tensor(out=alm[:, :], in0=al[:, :], in1=mu[:, :], op=ALU.mult)
                be = pp.tile([P, ncols], F32, tag=f"{tagp}_be")
                nc.vector.tensor_tensor(out=be[:, :], in0=bsb[:, :], in1=alm[:, :], op=ALU.subtract)
                return al, be

            al1, be1 = bn_coefs([s1sum0, s1sum1], [s1sq0, s1sq1], g1sb, b1sb, 2, "bn1")

            # ---- BN1+relu (DVE), layer-2 matmul (bf16) + stats ----------
            with tc.tile_pool(name="psD", bufs=2, space="PSUM") as psD:
                for g in range(NG):
                    gs = slice(g * GRP * P, (g + 1) * GRP * P)
                    for m, h1m in enumerate((h1a, h1b)):
                        nc.vector.tensor_scalar(
                            h1m[:, gs], h1m[:, gs],
                            al1[:, m : m + 1], be1[:, m : m + 1],
                            op0=ALU.mult, op1=ALU.add,
                        )
                        nc.vector.tensor_scalar_max(h1m[:, gs], h1m[:, gs], 0.0)
                    h2p = psD.tile([P, GRP * P], F32, tag="h2p")
                    nc.tensor.matmul(h2p, w20[:, :], h1a[:, gs], start=True, stop=False)
                    nc.tensor.matmul(h2p, w21[:, :], h1b[:, gs], start=False, stop=True)
                    nc.scalar.activation(
                        h2sb[:, gs], h2p, AF.Copy, accum_out=s2sum[:, g : g + 1]
                    )
                    sq2 = pw.tile([P, GRP * P], F32, tag="sqscratch")
                    nc.scalar.activation(
                        sq2, h2p, AF.Square, accum_out=s2sq[:, g : g + 1]
                    )

            al2, be2 = bn_coefs([s2sum], [s2sq], g2sb, b2sb, 1, "bn2")

            # ---- BN2+relu (DVE), store channel-major --------------------
            CH = 2048
            for c in range(N // CH):
                cs_ = slice(c * CH, (c + 1) * CH)
                nc.vector.tensor_scalar(
                    h2sb[:, cs_], h2sb[:, cs_],
                    al2[:, 0:1], be2[:, 0:1],
                    op0=ALU.mult, op1=ALU.add,
                )
                nc.vector.tensor_scalar_max(h2sb[:, cs_], h2sb[:, cs_], 0.0)
                nc.sync.dma_start(out_o[:, cs_], h2sb[:, cs_])

    nc.compile()
    return nc


_NC_CACHE = []


def _get_nc():
    if not _NC_CACHE:
        _NC_CACHE.append(_build())
    return _NC_CACHE[0]


def _split3(v):
    """3-way bf16 split of a float64 array: v ~= a + b + c exactly to ~2^-27."""
    a = v.astype(BFNP).astype(np.float64)
    b = (v - a).astype(BFNP).astype(np.float64)
    c = (v - a - b).astype(BFNP).astype(np.float64)
    return a, b, c


def _dist_rows(x1, x2):
    """Build the K=36 bf16 row pairs computing s = -||x1-x2||^2 fp32-exactly."""
    x = x1.astype(np.float64)
    u2 = x2.astype(np.float64)
    u = 2.0 * u2
    n, s = x.shape[0], u2.shape[0]
    ones_n = np.ones(n)
    ones_s = np.ones(s)
    L, R = [], []
    for i in range(3):
        a, b, c = _split3(x[:, i])
        d, e, f = _split3(u[:, i])
        s1, s2, s3 = _split3(-(x[:, i] ** 2))
        t1, t2, t3 = _split3(-(u2[:, i] ** 2))
        for l, r in [
            (a, d), (s1, ones_s), (ones_n, t1), (s2, ones_s), (ones_n, t2),
            (s3, ones_s), (ones_n, t3), (a, e), (b, d), (a, f), (b, e), (c, d),
        ]:
            L.append(l)
            R.append(r)
    A1b = np.stack(L).astype(BFNP)
    A2b = np.stack(R).astype(BFNP)
    return A1b, A2b


def _prep_inputs(xyz1, xyz2, points1, points2, W1, g1, b1, W2, g2, b2):
    xyz1 = np.asarray(xyz1, np.float32)
    xyz2 = np.asarray(xyz2, np.float32)
    points1 = np.asarray(points1, np.float32)
    points2 = np.asarray(points2, np.float32)
    W1 = np.asarray(W1, np.float32)
    W2 = np.asarray(W2, np.float32)
    W1T = np.ascontiguousarray(W1.T).astype(BFNP)  # [320, 256]
    W2T = np.ascontiguousarray(W2.T).astype(BFNP)  # [256, 128]
    shared = {
        "W1T0": W1T[0:D1],
        "W1T1": W1T[D1 : D1 + 128],
        "W1T2": W1T[D1 + 128 : D1 + 256],
        "W2T0": W2T[0:128],
        "W2T1": W2T[128:256],
        "g1p": np.ascontiguousarray(np.asarray(g1, np.float32).reshape(2, P).T),
        "b1p": np.ascontiguousarray(np.asarray(b1, np.float32).reshape(2, P).T),
        "g2p": np.ascontiguousarray(np.asarray(g2, np.float32).reshape(1, P).T),
        "b2p": np.ascontiguousarray(np.asarray(b2, np.float32).reshape(1, P).T),
    }
    in_maps = []
    for c in range(NCORES):
        A1b, A2b = _dist_rows(xyz1[c], xyz2[c])
        in_maps.append(
            {
                "A1b": A1b,
                "A2b": A2b,
                "p1T": np.ascontiguousarray(points1[c].T).astype(BFNP),
                "P2": np.ascontiguousarray(points2[c]).astype(BFNP),
                **shared,
            }
        )
    return in_maps


def run(inputs, trace=False, trace_kwargs=None):
    in_maps = _prep_inputs(**inputs)
    nc = _get_nc()
    res = run_bass_kernel_spmd(
        nc, in_maps, list(range(NCORES)), trace=trace, **(trace_kwargs or {})
    )
    # device output is channel-major [C2, N]; transpose per core on host
    out = np.stack(
        [np.ascontiguousarray(res.results[c]["out"].T) for c in range(NCORES)], axis=0
    )
    return out.astype(np.float32, copy=False), res


def kernel(**inputs) -> np.ndarray:
    out, _ = run(inputs, trace=False)
    return out
